# revision 17
# baseline (speedup 1.0000x reference)
"""Criss-cross attention (nn_CC_attention) Trainium2 kernel, v4.

Sharding: pure data parallel over batch B=8 across 8 NeuronCores; the only
cross-core coupling is the global min/max of energy. the two scalars are
exchanged via a tiny AllGather followed by a local max.

Host-side staging (layout/precision only; all model compute is on-device):
  t1b = fp8(tensor1)  as (H, C, W)   -- keys source
  t2h = fp16(tensor2) as (H, C, W)   -- carries the exact +tensor2 path
  t2t = fp8(tensor2)  as (W, C, H)   -- pre-transposed copy for the W branch
  out is produced as fp16 (H, C, W), host transposes back to (C, H, W) fp32.

Per-core device algorithm:
  phase 1 (stream 8 groups of 32 channels, loaded and processed as
  16-channel halves):
    kW[c][h,k] = sum-pool_w(t1)   (DVE reduce -> fp16; the 1/8 pool scale is
                                   dropped -- min-max normalization of the
                                   energies is scale-invariant)
    kH[c][w,k] = t1b[c].T @ P     (PE, t1 stationary; P = 0/1 pooling matrix)
    eW[w,k] += t2h[c].T @ kW[c]   (PE, t2h stationary, K=h, N=16)
    eH[h,k] += t2t[c].T @ kH[c]   (PE, t2t stationary, K=w, N=16)
  boundary:
    local (max,-min) over [128,32] energies -> partition all-reduce ->
    7x remote_dma_broadcast of the 2-scalar pack to all peers -> local max
    -> global range.  Meanwhile energies transpose to K-major [16, 256]
    (2 PE transposes), so exp/softmax and the attention matrices run on 16
    partitions with no post-softmax transpose:
      A_HT[h',h] = 0.0625*att_H[h,h'//8] + I (fp16)
      M_W[w',w]  = 0.0625*att_W[w,w'//8]     (bf16)
    (0.5 gamma and the two 1/8 value-pool scales fold into 0.0625; the full
     +tensor2 path rides A_HT's identity)
  phase 2 (16 groups of 16 channels):
    psum[h,(c,w)] = A_HT.T @ t2h[g]  (N=512 x4)
                  += t2t[c].T @ M_W  (per c, fp8 stationary)
    = 0.5*out_H + 0.5*out_W + tensor2 ;  ACT/DVE copy -> fp16 -> DMA out.
A PE warm-up burst (dummy matmuls) runs at kernel start so the HAM clock
gate is open during phase 1.
"""

import numpy as np
from contextlib import ExitStack

import ml_dtypes
import concourse.bass as bass
import concourse.tile as tile
from concourse import bacc, bass_isa, mybir

B, C, H, W, POOL = 8, 256, 128, 128, 8
KH, KW = H // POOL, W // POOL  # 16, 16
NCORES = 8
G = 32   # channels per load group
GH = 16  # channels per processing half
GO = 16  # channels per output group
NWARM = 56  # dummy matmuls in the kernel-start PE warm-up burst

F32 = mybir.dt.float32
F16 = mybir.dt.float16
BF16 = mybir.dt.bfloat16
F8 = mybir.dt.float8e4
BF_NP = ml_dtypes.bfloat16
F8_NP = ml_dtypes.float8_e4m3


def host_constants():
    pool_m = np.zeros((H, KH), np.float32)
    for k in range(KH):
        pool_m[k * POOL:(k + 1) * POOL, k] = 1.0  # no 1/8: norm is scale-inv
    expmat = np.zeros((KH, H), np.float32)
    for k in range(KH):
        expmat[k, k * POOL:(k + 1) * POOL] = 0.5 / POOL  # 0.0625
    return {
        "pool16": pool_m.astype(F8_NP),
        "ident16": np.eye(H, dtype=np.float32).astype(BF_NP),
        "expmat": expmat.astype(BF_NP),
        "eyefull": np.eye(H, dtype=np.float32),
    }


def build(c_total=C, ncores=NCORES):
    assert c_total % G == 0 and c_total % GO == 0
    ngroups = c_total // G
    nogroups = c_total // GO
    nc = bacc.Bacc(trn_type="TRN2", target_bir_lowering=False, debug=False,
                   num_devices=ncores)

    t1b = nc.dram_tensor("t1b", [H, c_total, W], F8, kind="ExternalInput").ap()
    t2h = nc.dram_tensor("t2h", [H, c_total, W], F16, kind="ExternalInput").ap()
    t2t = nc.dram_tensor("t2t", [W, c_total, H], F8, kind="ExternalInput").ap()
    pool16 = nc.dram_tensor("pool16", [H, KH], F8, kind="ExternalInput").ap()
    ident16 = nc.dram_tensor("ident16", [H, W], BF16, kind="ExternalInput").ap()
    expmat = nc.dram_tensor("expmat", [KH, H], BF16, kind="ExternalInput").ap()
    eyefull = nc.dram_tensor("eyefull", [H, W], F32, kind="ExternalInput").ap()
    out = nc.dram_tensor("out", [H, c_total, W], F16, kind="ExternalOutput").ap()

    with tile.TileContext(nc) as tc, ExitStack() as top:
        # ---- constants ----
        cpool = top.enter_context(tc.tile_pool(name="consts", bufs=1))
        c_pool16 = cpool.tile([H, KH], F8, tag="pool16")
        nc.sync.dma_start(c_pool16[:], pool16[:])
        c_ident = cpool.tile([H, W], BF16, tag="ident16")
        nc.sync.dma_start(c_ident[:], ident16[:])
        c_expmat = cpool.tile([KH, H], BF16, tag="expmat")
        nc.sync.dma_start(c_expmat[:], expmat[:])
        c_eye = cpool.tile([H, W], F32, tag="eyefull")
        nc.sync.dma_start(c_eye[:], eyefull[:])

        spool = top.enter_context(tc.tile_pool(name="soft", bufs=1))
        dram = top.enter_context(tc.tile_pool(name="dram", bufs=1, space="DRAM"))

        resq = top.enter_context(tc.tile_pool(name="resq", bufs=ngroups))
        resqT = top.enter_context(tc.tile_pool(name="resqT", bufs=ngroups))
        t2q_tiles, t2t_tiles = [], []

        psb = ExitStack()  # psum pools released before phase 2
        ps_e = psb.enter_context(tc.tile_pool(name="ps_e", bufs=1, space="PSUM"))
        ps_ehw = ps_e.tile([H, 2 * KH], F32, tag="ehw")  # cols 0:16 eH, 16:32 eW
        ps_et = ps_e.tile([KH, H + W], F32, tag="e")  # K-major: 0:H eH.T, H: eW.T
        ps_warm = psb.enter_context(tc.tile_pool(name="ps_warm", bufs=1, space="PSUM"))
        ps_w = ps_warm.tile([H, W], F32, tag="warm")

        # PE warm-up burst (no data deps -> scheduled at kernel start)
        for _ in range(NWARM):
            nc.tensor.matmul(ps_w[:], c_ident[:], c_ident[:], start=True, stop=True)

        # ================= phase 1 =================
        with ExitStack() as ph1:
            pin = ph1.enter_context(tc.tile_pool(name="pin", bufs=4))
            kpool = ph1.enter_context(tc.tile_pool(name="keys", bufs=6))
            ps_kh = ph1.enter_context(tc.tile_pool(name="ps_kh", bufs=4, space="PSUM"))

            for g in range(ngroups):
                c0 = g * G
                t1g = pin.tile([H, G * W], F8, tag="t1g")
                t2qg = resq.tile([H, G * W], F16, tag="t2qg")
                t2q_tiles.append(t2qg)
                t2tg = resqT.tile([W, G * H], F8, tag="t2tg")
                t2t_tiles.append(t2tg)
                for hf in range(G // GH):
                    ch0 = c0 + hf * GH
                    sl = slice(hf * GH * W, (hf + 1) * GH * W)
                    nc.scalar.dma_start(
                        t1g[:, sl].rearrange("p (c w) -> p c w", c=GH),
                        t1b[:, ch0:ch0 + GH, :])
                    nc.sync.dma_start(
                        t2qg[:, sl].rearrange("p (c w) -> p c w", c=GH),
                        t2h[:, ch0:ch0 + GH, :])
                    nc.sync.dma_start(
                        t2tg[:, sl].rearrange("p (c h) -> p c h", c=GH),
                        t2t[:, ch0:ch0 + GH, :])

                for hf in range(G // GH):
                    sl = slice(hf * GH * W, (hf + 1) * GH * W)
                    # kW[c][h,k] (fp16, unscaled sum-pool)
                    kW = kpool.tile([H, GH * KW], F16, tag="kW")
                    with nc.allow_low_precision(reason="pooled keys in fp16 "
                                                "feed min-max-normed energies"):
                        nc.vector.tensor_reduce(
                            kW[:].rearrange("p (c k) -> p c k", c=GH),
                            t1g[:, sl].rearrange("p (c k j) -> p c k j",
                                                 c=GH, j=POOL),
                            axis=mybir.AxisListType.X, op=mybir.AluOpType.add)

                    # kH[c][w,k] (bf16) = t1b[c].T @ pool16
                    ps_kh_t = ps_kh.tile([W, GH * KH], F32, tag="ps_kh")
                    for i in range(GH):
                        nc.tensor.matmul(ps_kh_t[:, i * KH:(i + 1) * KH],
                                         t1g[:, sl][:, i * W:(i + 1) * W],
                                         c_pool16[:], start=True, stop=True)
                    kH = kpool.tile([W, GH * KH], BF16, tag="kH")
                    nc.scalar.copy(kH[:], ps_kh_t[:])

                    first = (g == 0 and hf == 0)
                    last = (g == ngroups - 1 and hf == G // GH - 1)
                    for i in range(GH):
                        # eH[h,k] += t2t[c].T @ kH[c]
                        nc.tensor.matmul(ps_ehw[:, 0:KH],
                                         t2tg[:, sl][:, i * H:(i + 1) * H],
                                         kH[:, i * KH:(i + 1) * KH],
                                         start=(first and i == 0),
                                         stop=(last and i == GH - 1))
                        # eW[w,k] += t2h[c].T @ kW[c]
                        nc.tensor.matmul(ps_ehw[:, KH:2 * KH],
                                         t2qg[:, sl][:, i * W:(i + 1) * W],
                                         kW[:, i * KW:(i + 1) * KW],
                                         start=(first and i == 0),
                                         stop=(last and i == GH - 1))

        # ================= boundary =================
        # local (max, -min) over all 128 partitions of the H-major energies
        pack = spool.tile([H, 2], F32, tag="pack")
        nc.vector.tensor_reduce(pack[:, 0:1], ps_ehw[:], axis=mybir.AxisListType.X,
                                op=mybir.AluOpType.max)
        rmin = spool.tile([H, 1], F32, tag="rmin")
        nc.vector.tensor_reduce(rmin[:], ps_ehw[:], axis=mybir.AxisListType.X,
                                op=mybir.AluOpType.min)
        nc.vector.tensor_scalar_mul(pack[:, 1:2], rmin[:], -1.0)
        packr = spool.tile([H, 2], F32, tag="packr")
        nc.gpsimd.partition_all_reduce(packr[:], pack[:], channels=H,
                                       reduce_op=bass_isa.ReduceOp.max)

        # exchange the 2-scalar pack via a tiny AllGather, then local max
        cbuf = spool.tile([1, 16], F32, tag="cbuf")
        nc.vector.memset(cbuf[:], 0.0)
        nc.gpsimd.tensor_copy(cbuf[:, 0:2], packr[0:1, :])
        cc_in = dram.tile([1, 16], F32, tag="cc_in")
        cc_out = dram.tile([1, 16 * ncores], F32, tag="cc_out")
        nc.scalar.dma_start(cc_in[:], cbuf[:])
        nc.gpsimd.collective_compute(
            "AllGather", mybir.AluOpType.bypass,
            replica_groups=[list(range(ncores))],
            ins=[cc_in.opt()], outs=[cc_out.opt()],
        )

        # energies -> SBUF -> K-major [16, 256] via two PE transposes
        # (runs while the collective is in flight)
        e_sb = spool.tile([H, 2 * KH], F32, tag="e_sb")
        nc.vector.tensor_copy(e_sb[:], ps_ehw[:])
        nc.tensor.transpose(ps_et[:, 0:H], e_sb[:, 0:KH], c_eye[:])
        nc.tensor.transpose(ps_et[:, H:H + W], e_sb[:, KH:2 * KH], c_eye[:])

        g_all = spool.tile([1, 16 * ncores], F32, tag="g_all")
        nc.sync.dma_start(g_all[:], cc_out[:])
        gg = spool.tile([1, 16], F32, tag="gg")
        nc.vector.tensor_reduce(gg[:], g_all[:].rearrange("p (r s) -> p s r",
                                                          r=ncores),
                                axis=mybir.AxisListType.X, op=mybir.AluOpType.max)

        # softmax prep: compute (1/range, bias) on partition 0, broadcast to
        # the 16 energy partitions
        rng_t = spool.tile([1, 1], F32, tag="rng")
        nc.vector.tensor_tensor(rng_t[:], gg[:, 0:1], gg[:, 1:2],
                                mybir.AluOpType.add)
        iv1 = spool.tile([1, 2], F32, tag="iv1")
        nc.vector.reciprocal(iv1[:, 0:1], rng_t[:])
        nc.vector.tensor_tensor(iv1[:, 1:2], gg[:, 1:2], iv1[:, 0:1],
                                mybir.AluOpType.mult)
        ivb = spool.tile([KH, 2], F32, tag="ivb")
        nc.gpsimd.partition_broadcast(ivb[:], iv1[:])
        inv_t, bias_t = ivb[:, 0:1], ivb[:, 1:2]

        s_sb = spool.tile([KH, H + W], F32, tag="s_sb")
        ssum = spool.tile([KH, 1], F32, tag="ssum")
        nc.scalar.activation(s_sb[:], ps_et[:], mybir.ActivationFunctionType.Exp,
                             bias=bias_t, scale=inv_t, accum_out=ssum[:])
        stot = spool.tile([KH, 1], F32, tag="stot")
        nc.gpsimd.partition_all_reduce(stot[:], ssum[:], channels=KH,
                                       reduce_op=bass_isa.ReduceOp.add)
        rn = spool.tile([KH, 1], F32, tag="rn")
        nc.vector.reciprocal(rn[:], stot[:])
        s16 = spool.tile([KH, H + W], BF16, tag="s16")
        nc.vector.tensor_scalar_mul(s16[:], s_sb[:], rn[:])

        # A-mat builds (att already K-major; no transposes needed)
        apool = top.enter_context(tc.tile_pool(name="amats", bufs=1))
        with tc.tile_pool(name="ps_a", bufs=1, space="PSUM") as ps_a:
            ps_ah = ps_a.tile([H, H], F32, tag="ps_ah")
            nc.tensor.matmul(ps_ah[:], c_expmat[:], s16[:, 0:H], start=True, stop=True)
            A_HT = apool.tile([H, H], F16, tag="A_HT")
            nc.vector.scalar_tensor_tensor(A_HT[:], ps_ah[:], 1.0, c_eye[:],
                                           op0=mybir.AluOpType.mult,
                                           op1=mybir.AluOpType.add)
            ps_mw = ps_a.tile([W, W], F32, tag="ps_mw")
            nc.tensor.matmul(ps_mw[:], c_expmat[:], s16[:, H:H + W], start=True, stop=True)
            M_W = apool.tile([W, W], BF16, tag="M_W")
            nc.scalar.copy(M_W[:], ps_mw[:])

        psb.close()

        # ================= phase 2 =================
        with ExitStack() as ph2:
            ps_out = ph2.enter_context(tc.tile_pool(name="ps_out", bufs=2, space="PSUM"))
            opool = ph2.enter_context(tc.tile_pool(name="outp", bufs=4))
            sub = G // GO
            for og in range(nogroups):
                c0 = og * GO
                ld, hf = og // sub, og % sub
                t2q_sl = t2q_tiles[ld][:, hf * GO * W:(hf + 1) * GO * W]
                t2t_sl = t2t_tiles[ld][:, hf * GO * H:(hf + 1) * GO * H]
                ps_o = ps_out.tile([H, GO * W], F32, tag="ps_o")
                for j in range(0, GO * W, 512):
                    nc.tensor.matmul(ps_o[:, j:j + 512], A_HT[:], t2q_sl[:, j:j + 512],
                                     start=True, stop=False)
                for i in range(GO):
                    nc.tensor.matmul(ps_o[:, i * W:(i + 1) * W],
                                     t2t_sl[:, i * H:(i + 1) * H], M_W[:],
                                     start=False, stop=(i % 4 == 3))
                ob = opool.tile([H, GO * W], F16, tag="ob")
                half = GO * W // 2
                nc.scalar.copy(ob[:, 0:half], ps_o[:, 0:half])
                nc.sync.dma_start(out[:, c0:c0 + GO // 2, :],
                                  ob[:, 0:half].rearrange("p (c w) -> p c w", c=GO // 2))
                nc.vector.tensor_copy(ob[:, half:], ps_o[:, half:])
                nc.sync.dma_start(out[:, c0 + GO // 2:c0 + GO, :],
                                  ob[:, half:].rearrange("p (c w) -> p c w", c=GO // 2))

    nc.compile()
    return nc


_NC_CACHE = {}


def _get_nc():
    key = (C, NCORES)
    if key not in _NC_CACHE:
        _NC_CACHE[key] = build(C, NCORES)
    return _NC_CACHE[key]


def _stage(tensor1, tensor2):
    """Host-side precision/layout staging for all cores."""
    t1b = np.ascontiguousarray(
        tensor1.astype(F8_NP).transpose(0, 2, 1, 3))            # (B,H,C,W) fp8
    t2h = np.ascontiguousarray(
        tensor2.astype(np.float16).transpose(0, 2, 1, 3))       # (B,H,C,W) fp16
    t2t = np.ascontiguousarray(
        tensor2.astype(F8_NP).transpose(0, 3, 1, 2))            # (B,W,C,H) fp8
    return t1b, t2h, t2t


def kernel(tensor1: np.ndarray, tensor2: np.ndarray) -> np.ndarray:
    from concourse.bass_utils import run_bass_kernel_spmd
    assert tensor1.shape == (B, C, H, W) and tensor2.shape == (B, C, H, W)
    nc = _get_nc()
    consts = host_constants()
    t1b, t2h, t2t = _stage(np.asarray(tensor1, np.float32),
                           np.asarray(tensor2, np.float32))
    in_maps = [
        {"t1b": t1b[b], "t2h": t2h[b], "t2t": t2t[b], **consts}
        for b in range(B)
    ]
    res = run_bass_kernel_spmd(nc, in_maps, core_ids=list(range(NCORES)))
    out_hcw = np.stack([res.results[b]["out"] for b in range(B)])  # (B,H,C,W) f16
    return np.ascontiguousarray(
        out_hcw.transpose(0, 2, 1, 3).astype(np.float32))


# revision 22
# speedup vs baseline: 1.0228x; 1.0228x over previous
"""Criss-cross attention (nn_CC_attention) Trainium2 kernel, v5.

Sharding: pure data parallel over batch B=8 across 8 NeuronCores; the only
cross-core coupling is the global min/max of energy. The two scalars are
exchanged via a tiny AllGather followed by a local max.

Host-side staging (layout/precision only; all model compute is on-device):
  t1b = fp8(tensor1)  as (H, C, W)   -- keys source
  t2h = fp16(tensor2) as (H, C, W)   -- carries the exact +tensor2 path
  t2t = fp8(tensor2)  as (W, C, H)   -- pre-transposed copy for the W branch
  out is produced as fp16 (H, C, W), host transposes back to (C, H, W) fp32.

Per-core device algorithm:
  phase 1 (stream 8 groups of 32 channels, loaded and processed as
  16-channel halves):
    kW[c][h,k] = sum-pool_w(t1)   (DVE reduce -> fp16; the 1/8 pool scale is
                                   dropped -- min-max normalization of the
                                   energies is scale-invariant)
    kH[c][w,k] = t1b[c].T @ P     (PE, t1 stationary; P = 0/1 pooling matrix)
    eW[w,k] += t2h[c].T @ kW[c]   (PE, t2h stationary, K=h, N=16)
    eH[h,k] += t2t[c].T @ kH[c]   (PE, t2t stationary, K=w, N=16)
  boundary:
    local (max,-min) over [128,32] energies -> partition all-reduce ->
    AllGather(8x16 f32) -> local max -> global range.
    Meanwhile energies transpose to K-major [16, 256]
    (2 PE transposes), so exp/softmax and the attention matrices run on 16
    partitions with no post-softmax transpose:
      A_HT[h',h] = 0.0625*att_H[h,h'//8] + I (fp16)
      M_W[w',w]  = 0.0625*att_W[w,w'//8]     (bf16)
    (0.5 gamma and the two 1/8 value-pool scales fold into 0.0625; the full
     +tensor2 path rides A_HT's identity)
  phase 2 (16 groups of 16 channels):
    psum[h,(c,w)] = A_HT.T @ t2h[g]  (N=512 x4)
                  += t2t[c].T @ M_W  (per c, fp8 stationary)
    = 0.5*out_H + 0.5*out_W + tensor2 ;  ACT/DVE copy -> fp16 -> DMA out.
A PE warm-up burst (dummy matmuls) runs at kernel start so the HAM clock
gate is open during phase 1.
"""

import numpy as np
from contextlib import ExitStack

import ml_dtypes
import concourse.bass as bass
import concourse.tile as tile
from concourse import bacc, bass_isa, mybir

B, C, H, W, POOL = 8, 256, 128, 128, 8
KH, KW = H // POOL, W // POOL  # 16, 16
NCORES = 8
G = 32   # channels per load group
GH = 16  # channels per processing half
GO = 16  # channels per output group
NWARM = 56  # dummy matmuls in the kernel-start PE warm-up burst

F32 = mybir.dt.float32
F16 = mybir.dt.float16
BF16 = mybir.dt.bfloat16
F8 = mybir.dt.float8e4
BF_NP = ml_dtypes.bfloat16
F8_NP = ml_dtypes.float8_e4m3


def host_constants():
    pool_m = np.zeros((H, KH), np.float32)
    for k in range(KH):
        pool_m[k * POOL:(k + 1) * POOL, k] = 1.0  # no 1/8: norm is scale-inv
    expmat = np.zeros((KH, H), np.float32)
    for k in range(KH):
        expmat[k, k * POOL:(k + 1) * POOL] = 0.5 / POOL  # 0.0625
    return {
        "pool16": pool_m.astype(F8_NP),
        "ident16": np.eye(H, dtype=np.float32).astype(BF_NP),
        "expmat": expmat.astype(BF_NP),
        "eyefull": np.eye(H, dtype=np.float32),
    }


def build(c_total=C, ncores=NCORES):
    assert c_total % G == 0 and c_total % GO == 0
    ngroups = c_total // G
    nogroups = c_total // GO
    nc = bacc.Bacc(trn_type="TRN2", target_bir_lowering=False, debug=False,
                   num_devices=ncores)

    t1b = nc.dram_tensor("t1b", [H, c_total, W], F8, kind="ExternalInput").ap()
    t2h = nc.dram_tensor("t2h", [H, c_total, W], F16, kind="ExternalInput").ap()
    t2t = nc.dram_tensor("t2t", [W, c_total, H], F8, kind="ExternalInput").ap()
    pool16 = nc.dram_tensor("pool16", [H, KH], F8, kind="ExternalInput").ap()
    ident16 = nc.dram_tensor("ident16", [H, W], BF16, kind="ExternalInput").ap()
    expmat = nc.dram_tensor("expmat", [KH, H], BF16, kind="ExternalInput").ap()
    eyefull = nc.dram_tensor("eyefull", [H, W], F32, kind="ExternalInput").ap()
    out = nc.dram_tensor("out", [H, c_total, W], F16, kind="ExternalOutput").ap()

    with tile.TileContext(nc) as tc, ExitStack() as top:
        # ---- constants ----
        cpool = top.enter_context(tc.tile_pool(name="consts", bufs=1))
        c_pool16 = cpool.tile([H, KH], F8, tag="pool16")
        nc.sync.dma_start(c_pool16[:], pool16[:])
        c_ident = cpool.tile([H, W], BF16, tag="ident16")
        nc.sync.dma_start(c_ident[:], ident16[:])
        c_expmat = cpool.tile([KH, H], BF16, tag="expmat")
        nc.sync.dma_start(c_expmat[:], expmat[:])
        c_eye = cpool.tile([H, W], F32, tag="eyefull")
        nc.sync.dma_start(c_eye[:], eyefull[:])

        spool = top.enter_context(tc.tile_pool(name="soft", bufs=1))
        dram = top.enter_context(tc.tile_pool(name="dram", bufs=1, space="DRAM"))

        resq = top.enter_context(tc.tile_pool(name="resq", bufs=ngroups))
        resqT = top.enter_context(tc.tile_pool(name="resqT", bufs=ngroups))
        t2q_tiles, t2t_tiles = [], []

        psb = ExitStack()  # psum pools released before phase 2
        ps_e = psb.enter_context(tc.tile_pool(name="ps_e", bufs=1, space="PSUM"))
        ps_ehw = ps_e.tile([H, 2 * KH], F32, tag="ehw")  # cols 0:16 eH, 16:32 eW
        ps_et = ps_e.tile([KH, H + W], F32, tag="e")  # K-major: 0:H eH.T, H: eW.T
        ps_warm = psb.enter_context(tc.tile_pool(name="ps_warm", bufs=1, space="PSUM"))
        ps_w = ps_warm.tile([H, W], F32, tag="warm")

        # PE warm-up burst (no data deps -> scheduled at kernel start)
        for _ in range(NWARM):
            nc.tensor.matmul(ps_w[:], c_ident[:], c_ident[:], start=True, stop=True)

        # ================= phase 1 =================
        with ExitStack() as ph1:
            pin = ph1.enter_context(tc.tile_pool(name="pin", bufs=4))
            kpool = ph1.enter_context(tc.tile_pool(name="keys", bufs=6))
            ps_kh = ph1.enter_context(tc.tile_pool(name="ps_kh", bufs=4, space="PSUM"))

            for g in range(ngroups):
                c0 = g * G
                t1g = pin.tile([H, G * W], F8, tag="t1g")
                t2qg = resq.tile([H, G * W], F16, tag="t2qg")
                t2q_tiles.append(t2qg)
                t2tg = resqT.tile([W, G * H], F8, tag="t2tg")
                t2t_tiles.append(t2tg)
                nc.scalar.dma_start(t1g[:].rearrange("p (c w) -> p c w", c=G),
                                    t1b[:, c0:c0 + G, :])
                nc.sync.dma_start(t2qg[:].rearrange("p (c w) -> p c w", c=G),
                                  t2h[:, c0:c0 + G, :])
                nc.sync.dma_start(t2tg[:].rearrange("p (c h) -> p c h", c=G),
                                  t2t[:, c0:c0 + G, :])

                for hf in range(G // GH):
                    sl = slice(hf * GH * W, (hf + 1) * GH * W)
                    # kW[c][h,k] (fp16, unscaled sum-pool)
                    kW = kpool.tile([H, GH * KW], F16, tag="kW")
                    with nc.allow_low_precision(reason="pooled keys in fp16 "
                                                "feed min-max-normed energies"):
                        nc.vector.tensor_reduce(
                            kW[:].rearrange("p (c k) -> p c k", c=GH),
                            t1g[:, sl].rearrange("p (c k j) -> p c k j",
                                                 c=GH, j=POOL),
                            axis=mybir.AxisListType.X, op=mybir.AluOpType.add)

                    # kH[c][w,k] (bf16) = t1b[c].T @ pool16
                    ps_kh_t = ps_kh.tile([W, GH * KH], F32, tag="ps_kh")
                    for i in range(GH):
                        nc.tensor.matmul(ps_kh_t[:, i * KH:(i + 1) * KH],
                                         t1g[:, sl][:, i * W:(i + 1) * W],
                                         c_pool16[:], start=True, stop=True)
                    kH = kpool.tile([W, GH * KH], BF16, tag="kH")
                    nc.scalar.copy(kH[:], ps_kh_t[:])

                    first = (g == 0 and hf == 0)
                    last = (g == ngroups - 1 and hf == G // GH - 1)
                    for i in range(GH):
                        # eH[h,k] += t2t[c].T @ kH[c]
                        nc.tensor.matmul(ps_ehw[:, 0:KH],
                                         t2tg[:, sl][:, i * H:(i + 1) * H],
                                         kH[:, i * KH:(i + 1) * KH],
                                         start=(first and i == 0),
                                         stop=(last and i == GH - 1))
                        # eW[w,k] += t2h[c].T @ kW[c]
                        nc.tensor.matmul(ps_ehw[:, KH:2 * KH],
                                         t2qg[:, sl][:, i * W:(i + 1) * W],
                                         kW[:, i * KW:(i + 1) * KW],
                                         start=(first and i == 0),
                                         stop=(last and i == GH - 1))

        # ================= boundary =================
        # local (max, -min) over all 128 partitions of the H-major energies
        pack = spool.tile([H, 2], F32, tag="pack")
        nc.vector.tensor_reduce(pack[:, 0:1], ps_ehw[:], axis=mybir.AxisListType.X,
                                op=mybir.AluOpType.max)
        rmin = spool.tile([H, 1], F32, tag="rmin")
        nc.vector.tensor_reduce(rmin[:], ps_ehw[:], axis=mybir.AxisListType.X,
                                op=mybir.AluOpType.min)
        nc.vector.tensor_scalar_mul(pack[:, 1:2], rmin[:], -1.0)
        packr = spool.tile([H, 2], F32, tag="packr")
        nc.gpsimd.partition_all_reduce(packr[:], pack[:], channels=H,
                                       reduce_op=bass_isa.ReduceOp.max)

        # exchange the 2-scalar pack via a tiny AllGather, then local max
        cbuf = spool.tile([1, 16], F32, tag="cbuf")
        nc.vector.memset(cbuf[:], 0.0)
        nc.gpsimd.tensor_copy(cbuf[:, 0:2], packr[0:1, :])
        cc_in = dram.tile([1, 16], F32, tag="cc_in")
        cc_out = dram.tile([1, 16 * ncores], F32, tag="cc_out")
        nc.scalar.dma_start(cc_in[:], cbuf[:])
        nc.gpsimd.collective_compute(
            "AllGather", mybir.AluOpType.bypass,
            replica_groups=[list(range(ncores))],
            ins=[cc_in.opt()], outs=[cc_out.opt()],
        )

        # energies -> SBUF -> K-major [16, 256] via two PE transposes
        # (runs while the collective is in flight)
        e_sb = spool.tile([H, 2 * KH], F32, tag="e_sb")
        nc.vector.tensor_copy(e_sb[:], ps_ehw[:])
        nc.tensor.transpose(ps_et[:, 0:H], e_sb[:, 0:KH], c_eye[:])
        nc.tensor.transpose(ps_et[:, H:H + W], e_sb[:, KH:2 * KH], c_eye[:])

        g_all = spool.tile([1, 16 * ncores], F32, tag="g_all")
        nc.sync.dma_start(g_all[:], cc_out[:])

        # PE warm-up burst keyed on the collective result: spins PE through
        # the softmax chain so the A-mat builds and phase 2 start warm
        wtile = spool.tile([H, W], BF16, tag="wtile")
        nc.vector.memset(wtile[:], 0.0)
        g2b = spool.tile([1, 1], BF16, tag="g2b")
        nc.vector.tensor_copy(g2b[:], g_all[:, 0:1])
        nc.gpsimd.partition_broadcast(wtile[:, 0:1], g2b[:])
        for _ in range(32):
            nc.tensor.matmul(ps_w[:], c_ident[:], wtile[:], start=True, stop=True)

        gg = spool.tile([1, 16], F32, tag="gg")
        nc.vector.tensor_reduce(gg[:], g_all[:].rearrange("p (r s) -> p s r",
                                                          r=ncores),
                                axis=mybir.AxisListType.X, op=mybir.AluOpType.max)

        # softmax prep: compute (1/range, bias) on partition 0, broadcast to
        # the 16 energy partitions
        rng_t = spool.tile([1, 1], F32, tag="rng")
        nc.vector.tensor_tensor(rng_t[:], gg[:, 0:1], gg[:, 1:2],
                                mybir.AluOpType.add)
        iv1 = spool.tile([1, 2], F32, tag="iv1")
        nc.vector.reciprocal(iv1[:, 0:1], rng_t[:])
        nc.vector.tensor_tensor(iv1[:, 1:2], gg[:, 1:2], iv1[:, 0:1],
                                mybir.AluOpType.mult)
        ivb = spool.tile([KH, 2], F32, tag="ivb")
        nc.gpsimd.partition_broadcast(ivb[:], iv1[:])
        inv_t, bias_t = ivb[:, 0:1], ivb[:, 1:2]

        s_sb = spool.tile([KH, H + W], F32, tag="s_sb")
        ssum = spool.tile([KH, 1], F32, tag="ssum")
        nc.scalar.activation(s_sb[:], ps_et[:], mybir.ActivationFunctionType.Exp,
                             bias=bias_t, scale=inv_t, accum_out=ssum[:])
        stot = spool.tile([KH, 1], F32, tag="stot")
        nc.gpsimd.partition_all_reduce(stot[:], ssum[:], channels=KH,
                                       reduce_op=bass_isa.ReduceOp.add)
        rn = spool.tile([KH, 1], F32, tag="rn")
        nc.vector.reciprocal(rn[:], stot[:])
        s16 = spool.tile([KH, H + W], BF16, tag="s16")
        nc.vector.tensor_scalar_mul(s16[:], s_sb[:], rn[:])

        # A-mat builds (att already K-major; no transposes needed)
        apool = top.enter_context(tc.tile_pool(name="amats", bufs=1))
        with tc.tile_pool(name="ps_a", bufs=1, space="PSUM") as ps_a:
            ps_ah = ps_a.tile([H, H], F32, tag="ps_ah")
            nc.tensor.matmul(ps_ah[:], c_expmat[:], s16[:, 0:H], start=True, stop=True)
            A_HT = apool.tile([H, H], F16, tag="A_HT")
            nc.vector.scalar_tensor_tensor(A_HT[:], ps_ah[:], 1.0, c_eye[:],
                                           op0=mybir.AluOpType.mult,
                                           op1=mybir.AluOpType.add)
            ps_mw = ps_a.tile([W, W], F32, tag="ps_mw")
            nc.tensor.matmul(ps_mw[:], c_expmat[:], s16[:, H:H + W], start=True, stop=True)
            M_W = apool.tile([W, W], BF16, tag="M_W")
            nc.scalar.copy(M_W[:], ps_mw[:])

        psb.close()

        # ================= phase 2 =================
        with ExitStack() as ph2:
            ps_out = ph2.enter_context(tc.tile_pool(name="ps_out", bufs=2, space="PSUM"))
            opool = ph2.enter_context(tc.tile_pool(name="outp", bufs=4))
            sub = G // GO
            for og in range(nogroups):
                c0 = og * GO
                ld, hf = og // sub, og % sub
                t2q_sl = t2q_tiles[ld][:, hf * GO * W:(hf + 1) * GO * W]
                t2t_sl = t2t_tiles[ld][:, hf * GO * H:(hf + 1) * GO * H]
                ps_o = ps_out.tile([H, GO * W], F32, tag="ps_o")
                for j in range(0, GO * W, 512):
                    nc.tensor.matmul(ps_o[:, j:j + 512], A_HT[:], t2q_sl[:, j:j + 512],
                                     start=True, stop=False)
                for i in range(GO):
                    nc.tensor.matmul(ps_o[:, i * W:(i + 1) * W],
                                     t2t_sl[:, i * H:(i + 1) * H], M_W[:],
                                     start=False, stop=(i % 4 == 3))
                # 4-way copy split (alternating ACT/DVE) releases each PSUM
                # bank as soon as its accumulation stops; 2 output DMAs
                ob = opool.tile([H, GO * W], F16, tag="ob")
                for q in range(4):
                    c4 = slice(q * 512, (q + 1) * 512)
                    if q % 2 == 0:
                        nc.scalar.copy(ob[:, c4], ps_o[:, c4])
                    else:
                        nc.vector.tensor_copy(ob[:, c4], ps_o[:, c4])
                half = GO * W // 2
                nc.sync.dma_start(out[:, c0:c0 + GO // 2, :],
                                  ob[:, 0:half].rearrange("p (c w) -> p c w", c=GO // 2))
                nc.sync.dma_start(out[:, c0 + GO // 2:c0 + GO, :],
                                  ob[:, half:].rearrange("p (c w) -> p c w", c=GO // 2))

    nc.compile()
    return nc


_NC_CACHE = {}


def _get_nc():
    key = (C, NCORES)
    if key not in _NC_CACHE:
        _NC_CACHE[key] = build(C, NCORES)
    return _NC_CACHE[key]


def _stage(tensor1, tensor2):
    """Host-side precision/layout staging for all cores."""
    t1b = np.ascontiguousarray(
        tensor1.astype(F8_NP).transpose(0, 2, 1, 3))            # (B,H,C,W) fp8
    t2h = np.ascontiguousarray(
        tensor2.astype(np.float16).transpose(0, 2, 1, 3))       # (B,H,C,W) fp16
    t2t = np.ascontiguousarray(
        tensor2.astype(F8_NP).transpose(0, 3, 1, 2))            # (B,W,C,H) fp8
    return t1b, t2h, t2t


def kernel(tensor1: np.ndarray, tensor2: np.ndarray) -> np.ndarray:
    from concourse.bass_utils import run_bass_kernel_spmd
    assert tensor1.shape == (B, C, H, W) and tensor2.shape == (B, C, H, W)
    nc = _get_nc()
    consts = host_constants()
    t1b, t2h, t2t = _stage(np.asarray(tensor1, np.float32),
                           np.asarray(tensor2, np.float32))
    in_maps = [
        {"t1b": t1b[b], "t2h": t2h[b], "t2t": t2t[b], **consts}
        for b in range(B)
    ]
    res = run_bass_kernel_spmd(nc, in_maps, core_ids=list(range(NCORES)))
    out_hcw = np.stack([res.results[b]["out"] for b in range(B)])  # (B,H,C,W) f16
    return np.ascontiguousarray(
        out_hcw.transpose(0, 2, 1, 3).astype(np.float32))
